# revision 6
# baseline (speedup 1.0000x reference)
"""BiAttentionMRU Trainium2 kernel.

Data-parallel over batch: B=16 -> 2 batch elements on each of 8 cores.
All weights replicated. Embedding gather done on-device via indirect DMA.

Layouts (per core, per batch element b in {0,1}):
  art gathered as [t-chunk(128), d=300] (tile padded to 384 cols), then
  DMA-xbar-transposed into artT[dc] tiles with d on partitions in chunks of
  (128, 128, 44). Group sums, z/o/CE matmuls, gate mix (all on PE as
  scaled-identity accumulating matmuls), MRU scan (native tensor_tensor_scan
  along t) and the attention lhsT all work in [d, t].

Attention algebra: aoq is never materialized. With e1 = exp(art_enc @ keys1^T),
Z1 its row sum (computed as an extra ones-column of the QK matmul),
s2 = softmax-normalized p1 @ (q @ keys_f^T) is computed as exp-of(u2 * 1/Z1)
where u2 = e1 @ QK. The per-option mean over t of softmax(s2) @ opt folds into
accumulating matmuls sum_t e2[t,:] * (1/Z2[t]).

All weights ship as two packed DRAM tensors (one bf16, one f32) and all
indices as one packed i32 tensor, loaded with three DMAs; phases are emitted
interleaved across the two batch elements so the scheduler can keep PE dense
while DVE runs scans/reductions.
"""

import sys

sys.path.insert(0, "/opt/trn_rl_repo")

import numpy as np
import ml_dtypes

import concourse.bass as bass
import concourse.tile as tile
from concourse import bacc, mybir
from concourse.masks import make_identity

F32 = mybir.dt.float32
BF16 = mybir.dt.bfloat16
I32 = mybir.dt.int32
AX = mybir.AxisListType
OP = mybir.AluOpType
AF = mybir.ActivationFunctionType

DIM = 300
VOCAB = 50000
B_FULL = 16
NCORES = 8
BPC = B_FULL // NCORES  # batch per core = 2
T = 2000
TQ = 30
TO = 16
RANGES = (1, 2, 4, 10, 25)

TCH = [128] * 15 + [80]  # t chunking for gathers/transposes
NTCH = len(TCH)
DC = 3
DCH = [128, 128, 44]     # d-chunk partition sizes
DOFF = [0, 128, 256]

# 512-col chunking for the z/o/B1 streams (bank-aligned psum halves)
T512 = [(0, 512), (512, 512), (1024, 512), (1536, 464)]
TSUB = [128, 128, 128, 128]
TSUB_LAST = [128, 128, 128, 80]

N_MM = 500  # gate-mix chunking: divisible by lcm(2,4,10,25)=100

USE_BF16 = True
DT = BF16 if USE_BF16 else F32
NPDT = ml_dtypes.bfloat16 if USE_BF16 else np.float32

# ---- packed bf16 weights: [128, 9000]; kc block holds contraction rows
#      DOFF[kc]..DOFF[kc]+DCH[kc] (rows above DCH[kc] are zero) ----
W_ART = 0          # 3 kc * 900 (z|o|ce0 blocks of 300)
W_CE = 2700        # 3 kc * 4 ri * 300
W_F1 = 6300        # 3 kc * 300
W_F2 = 7200
W_F3 = 8100
WCOLS = 9000

# ---- packed f32 tensor: [128, 506] ----
F_BIAS = 0         # kc*10 + {0 bz, 1 bo, 2..6 ce_b, 7 f1_b, 8 f2_b, 9 f3_b}
F_SCAL = 30        # 24 scalar cols (below)
F_AS1 = 54         # 6 blocks of 75; row counts (128,128,44,128,128,44)
F_AS2 = 504        # rows 0..74
F_BAS1 = 505       # rows 0..74
FCOLS = 506
AS_SZ = [128, 128, 44, 128, 128, 44]

SC_M1 = 0      # 15 cols: m1[k,r]/r at 5k+ri
SC_M1B = 15    # 3 cols
SC_M2 = 18     # 3 cols
SC_M2B = 21    # 1 col
SC_AS2B = 22   # 1 col

# ---- packed i32 indices: [128, BPC, 21] ----
# cols 0..14: art chunks (128 rows), 15: art tail (80), 16: q (30),
# 17..20: option o (16 rows)
IX_ART = 0
IX_Q = 16
IX_OPT = 17
IXCOLS = 21


def _build_program():
    nc = bacc.Bacc("TRN2", target_bir_lowering=False, debug=False,
                   num_devices=NCORES)

    emb = nc.dram_tensor("emb", [VOCAB, DIM], DT, kind="ExternalInput")
    idx_pack = nc.dram_tensor("idx_pack", [128, BPC, IXCOLS], I32,
                              kind="ExternalInput")
    wpack = nc.dram_tensor("wpack", [128, WCOLS], DT, kind="ExternalInput")
    fpack = nc.dram_tensor("fpack", [128, FCOLS], F32, kind="ExternalInput")
    out = nc.dram_tensor("scores", [BPC, 4], F32, kind="ExternalOutput")

    with tile.TileContext(nc) as tc:
        from contextlib import ExitStack
        with ExitStack() as ctx:
            _emit(nc, tc, ctx, emb, idx_pack, wpack, fpack, out)

    nc.compile()
    return nc


def _emit(nc, tc, ctx, emb, idx_pack, wpack, fpack, out):
    # ---------------- pools ----------------
    consts = ctx.enter_context(tc.tile_pool(name="consts", bufs=1))
    gpool = ctx.enter_context(tc.tile_pool(name="gather", bufs=4))
    p_art = ctx.enter_context(tc.tile_pool(name="p_art", bufs=2))
    p_enc = ctx.enter_context(tc.tile_pool(name="p_enc", bufs=2))
    p_zb = ctx.enter_context(tc.tile_pool(name="p_zb", bufs=2))
    p_mix = ctx.enter_context(tc.tile_pool(name="p_mix", bufs=2))
    p_xs = ctx.enter_context(tc.tile_pool(name="p_xs", bufs=1))
    persist = ctx.enter_context(tc.tile_pool(name="persist", bufs=1))
    small = ctx.enter_context(tc.tile_pool(name="small", bufs=4))
    # PSUM budget (8 banks): zo 2x2-bank + gate 2x1 + work 2x1
    pp_zo = ctx.enter_context(tc.tile_pool(name="pp_zo", bufs=2, space="PSUM"))
    pp_g = ctx.enter_context(tc.tile_pool(name="pp_g", bufs=2, space="PSUM"))
    pp_w = ctx.enter_context(tc.tile_pool(name="pp_w", bufs=2, space="PSUM"))

    # ---------------- constants / weights (3 packed DMAs) ----------------
    ixp = consts.tile([128, BPC, IXCOLS], I32)
    nc.sync.dma_start(ixp[:], idx_pack[:])
    wp = consts.tile([128, WCOLS], DT)
    nc.sync.dma_start(wp[:], wpack[:])
    fp = consts.tile([128, FCOLS], F32)
    nc.sync.dma_start(fp[:], fpack[:])

    def w_art_v(kc, mcol, mw):     # [DCH[kc], mw] block of z|o|ce0 weights
        return wp[0:DCH[kc], W_ART + kc * 900 + mcol:W_ART + kc * 900 + mcol + mw]

    def w_ce_v(kc, ri, dc):        # ranges 2,4,10,25
        o = W_CE + kc * 1200 + ri * 300 + DOFF[dc]
        return wp[0:DCH[kc], o:o + DCH[dc]]

    def w_f_v(base, kc, c0=0, cw=DIM):
        return wp[0:DCH[kc], base + kc * 300 + c0:base + kc * 300 + c0 + cw]

    def bias(dc, col):             # [DCH[dc], 1] f32
        return fp[0:DCH[dc], dc * 10 + col:dc * 10 + col + 1]

    def sc(col, p=128):            # f32 per-partition scalar
        return fp[0:p, F_SCAL + col:F_SCAL + col + 1]

    ident = consts.tile([128, 128], DT)
    make_identity(nc, ident[:])

    # scaled 128x128 identities for the PE-side gate mix:
    # cols j=5k+ri hold m1[k,ri]/r * I, cols 15+k hold m2[k] * I
    mI = consts.tile([128, 18, 128], DT)
    for j in range(18):
        scol = (SC_M1 + j) if j < 15 else (SC_M2 + j - 15)
        nc.vector.tensor_scalar_mul(mI[:, j, :], ident[:], sc(scol))

    ans_sb = persist.tile([128, BPC, 6, 4], F32, tag="ans_sb")

    # ---------------- gathers + DMA-xbar transposes (both batches) -------
    gathered = []
    for b in range(BPC):
        artT = [p_art.tile([128, T], DT, tag=f"artT{dc}", name=f"artT{dc}")
                for dc in range(DC)]
        # per-chunk indirect gathers (one 128-row gather per issue; the
        # batched multi-column offset-AP form mis-gathers on real HW).
        # Gather tile is 384 cols so the three 128-wide xbar transposes are
        # legal; cols 300..384 transpose into artT rows >= 44, never read.
        for c in range(NTCH):
            pc = TCH[c]
            g = gpool.tile([128, 384], DT, tag="gart", name="gart")
            nc.gpsimd.indirect_dma_start(
                out=g[:pc, 0:DIM], out_offset=None, in_=emb[:],
                in_offset=bass.IndirectOffsetOnAxis(
                    ap=ixp[:pc, b, IX_ART + c:IX_ART + c + 1], axis=0))
            for dc in range(DC):
                tp = pp_w.tile([128, 128], DT, tag="w")
                nc.tensor.transpose(tp[:, :pc], g[:pc, DOFF[dc]:DOFF[dc] + 128],
                                    ident[:pc, :pc])
                nc.vector.tensor_copy(artT[dc][0:DCH[dc], c * 128:c * 128 + pc],
                                      tp[0:DCH[dc], :pc])

        qg = persist.tile([TQ, DIM], DT, tag=f"qg{b}")
        nc.gpsimd.indirect_dma_start(
            out=qg[:], out_offset=None, in_=emb[:],
            in_offset=bass.IndirectOffsetOnAxis(
                ap=ixp[0:TQ, b, IX_Q:IX_Q + 1], axis=0))
        qT = persist.tile([128, DC, TQ], DT, tag=f"qT{b}")
        for dc in range(DC):
            tp = pp_w.tile([128, 128], DT, tag="w")
            nc.tensor.transpose(tp[0:DCH[dc], :TQ],
                                qg[:, DOFF[dc]:DOFF[dc] + DCH[dc]],
                                ident[:TQ, :TQ])
            nc.vector.tensor_copy(qT[0:DCH[dc], dc, :], tp[0:DCH[dc], :TQ])

        og = [persist.tile([TO, DIM], DT, tag=f"og{o}_{b}", name=f"og{o}")
              for o in range(4)]
        oT = persist.tile([128, DC, 4, TO], DT, tag=f"oT{b}")
        for o in range(4):
            nc.gpsimd.indirect_dma_start(
                out=og[o][:], out_offset=None, in_=emb[:],
                in_offset=bass.IndirectOffsetOnAxis(
                    ap=ixp[0:TO, b, IX_OPT + o:IX_OPT + o + 1], axis=0))
            for dc in range(DC):
                tp = pp_w.tile([128, 128], DT, tag="w")
                nc.tensor.transpose(tp[0:DCH[dc], :TO],
                                    og[o][:, DOFF[dc]:DOFF[dc] + DCH[dc]],
                                    ident[:TO, :TO])
                nc.vector.tensor_copy(oT[0:DCH[dc], dc, o, :],
                                      tp[0:DCH[dc], :TO])

        gathered.append(dict(artT=artT, qg=qg, qT=qT, og=og, oT=oT))

    # ---------------- group sums (xs_r in [d, g] layout) ----------------
    xs = [None] * BPC
    for b in range(BPC):
        artT = gathered[b]["artT"]
        xs2 = [p_xs.tile([128, T // 2], DT, tag=f"xs2_{dc}_{b}", name="xs2") for dc in range(DC)]
        xs4 = [p_xs.tile([128, T // 4], DT, tag=f"xs4_{dc}_{b}", name="xs4") for dc in range(DC)]
        xs10 = [p_xs.tile([128, T // 10], DT, tag=f"xs10_{dc}_{b}", name="xs10") for dc in range(DC)]
        xs25 = [p_xs.tile([128, T // 25], DT, tag=f"xs25_{dc}_{b}", name="xs25") for dc in range(DC)]
        for dc in range(DC):
            p = DCH[dc]
            a = artT[dc]
            nc.vector.tensor_add(xs2[dc][0:p, :], a[0:p, 0:T:2], a[0:p, 1:T:2])
            nc.gpsimd.tensor_add(xs4[dc][0:p, :], xs2[dc][0:p, 0:T // 2:2],
                                 xs2[dc][0:p, 1:T // 2:2])
            with nc.allow_low_precision(reason="bf16 group sums feed bf16 matmuls"):
                nc.vector.tensor_reduce(
                    xs10[dc][0:p, :],
                    xs2[dc][0:p, :].rearrange("p (g r) -> p g r", r=5),
                    AX.X, OP.add)
                nc.vector.tensor_reduce(
                    xs25[dc][0:p, :],
                    a[0:p, :].rearrange("p (g r) -> p g r", r=25),
                    AX.X, OP.add)
        xs[b] = dict(xs2=xs2, xs4=xs4, xs10=xs10, xs25=xs25)

    # ---------------- z / o / B1 (art stream, 512-chunks, 2-bank ACTs) ----
    zob = [None] * BPC
    for b in range(BPC):
        artT = gathered[b]["artT"]
        z_sb = [p_zb.tile([128, T], DT, tag=f"z{dc}", name=f"z{dc}") for dc in range(DC)]
        o_sb = [p_enc.tile([128, T], DT, tag=f"o{dc}", name=f"o{dc}") for dc in range(DC)]
        b1_sb = [p_zb.tile([128, T], DT, tag=f"b1_{dc}", name=f"b1_{dc}") for dc in range(DC)]
        for mi, (dst, func, bcol) in enumerate(
                ((z_sb, AF.Tanh, 0), (o_sb, AF.Tanh, 1), (b1_sb, AF.Relu, 2))):
            for dc in range(DC):
                mcol = mi * DIM + DOFF[dc]
                for pair in ((0, 1), (2, 3)):
                    ps = pp_zo.tile([128, 1024], F32, tag="zo")
                    for half, ci in enumerate(pair):
                        t0, tn = T512[ci]
                        for kc in range(DC):
                            nc.tensor.matmul(
                                ps[0:DCH[dc], half * 512:half * 512 + tn],
                                w_art_v(kc, mcol, DCH[dc]),
                                artT[kc][0:DCH[kc], t0:t0 + tn],
                                start=(kc == 0), stop=(kc == DC - 1))
                    t0 = T512[pair[0]][0]
                    tw = T512[pair[0]][1] + T512[pair[1]][1]
                    nc.scalar.activation(dst[dc][0:DCH[dc], t0:t0 + tw],
                                         ps[0:DCH[dc], 0:tw],
                                         func, bias=bias(dc, bcol))
        zob[b] = dict(z=z_sb, o=o_sb, b1=b1_sb)

    # ---------------- CE r>=2 ----------------
    bls = [None] * BPC
    for b in range(BPC):
        x = xs[b]
        bl = {}
        for ri, (xsr, r) in enumerate(((x["xs2"], 2), (x["xs4"], 4),
                                       (x["xs10"], 10), (x["xs25"], 25))):
            g_r = T // r
            bl[r] = [p_xs.tile([128, g_r], DT, tag=f"bl{r}_{dc}_{b}", name="bl")
                     for dc in range(DC)]
            for dc in range(DC):
                p = DCH[dc]
                if g_r > 512:  # r=2: two bank-halves, one ACT
                    ps = pp_zo.tile([128, 1024], F32, tag="zo")
                    for half, (g0, gn) in enumerate(((0, 512), (512, g_r - 512))):
                        for kc in range(DC):
                            nc.tensor.matmul(
                                ps[0:p, half * 512:half * 512 + gn],
                                w_ce_v(kc, ri, dc), xsr[kc][0:DCH[kc], g0:g0 + gn],
                                start=(kc == 0), stop=(kc == DC - 1))
                    nc.scalar.activation(bl[r][dc][0:p, :], ps[0:p, 0:g_r],
                                         AF.Relu, bias=bias(dc, 3 + ri))
                else:
                    ps = pp_g.tile([128, 512], F32, tag="g")
                    for kc in range(DC):
                        nc.tensor.matmul(ps[0:p, :g_r], w_ce_v(kc, ri, dc),
                                         xsr[kc][0:DCH[kc], :], start=(kc == 0),
                                         stop=(kc == DC - 1))
                    nc.scalar.activation(bl[r][dc][0:p, :], ps[0:p, :g_r],
                                         AF.Relu, bias=bias(dc, 3 + ri))
        bls[b] = bl

    # ---------------- attention prep: keys1T, A2/A3, QK (+ones col) -------
    attp = [None] * BPC
    for b in range(BPC):
        qT = gathered[b]["qT"]
        oT = gathered[b]["oT"]
        k1T = persist.tile([128, DC, TQ], DT, tag=f"k1T{b}")
        for dc in range(DC):
            ps = pp_w.tile([128, TQ], F32, tag="w")
            for kc in range(DC):
                nc.tensor.matmul(ps[0:DCH[dc], :],
                                 w_f_v(W_F1, kc, DOFF[dc], DCH[dc]),
                                 qT[0:DCH[kc], kc, :],
                                 start=(kc == 0), stop=(kc == DC - 1))
            nc.scalar.copy(k1T[0:DCH[dc], dc, :], ps[0:DCH[dc], :])

        aTs = []
        for fi, base in enumerate((W_F2, W_F3)):
            a_ps = pp_w.tile([TQ, DIM], F32, tag="w")
            for kc in range(DC):
                nc.tensor.matmul(a_ps[:], qT[0:DCH[kc], kc, :], w_f_v(base, kc),
                                 start=(kc == 0), stop=(kc == DC - 1))
            a_sb = persist.tile([TQ, DIM], DT, tag=f"a_sb{b}")
            nc.vector.tensor_copy(a_sb[:], a_ps[:])
            aT = persist.tile([128, DC, TQ], DT, tag=f"aT{fi}_{b}")
            for dc in range(DC):
                tp = pp_w.tile([128, 128], DT, tag="w")
                nc.tensor.transpose(tp[0:DCH[dc], :TQ],
                                    a_sb[:, DOFF[dc]:DOFF[dc] + DCH[dc]],
                                    ident[:TQ, :TQ])
                nc.vector.tensor_copy(aT[0:DCH[dc], dc, :], tp[0:DCH[dc], :TQ])
            aTs.append(aT)

        # qk[w-slot, 16*(4fi+o)+w] plus col 128 = 1.0 (gives Z1 via u2)
        qk_ps = pp_w.tile([TQ, 128], F32, tag="w")
        for fi in range(2):
            for kc in range(DC):
                nc.tensor.matmul(
                    qk_ps[:, 64 * fi:64 * fi + 64],
                    aTs[fi][0:DCH[kc], kc, :],
                    oT[0:DCH[kc], kc, :, :].rearrange("p o w -> p (o w)"),
                    start=(kc == 0), stop=(kc == DC - 1))
        qk_sb = persist.tile([TQ, 132], DT, tag=f"qk_sb{b}")
        nc.vector.tensor_copy(qk_sb[:, 0:128], qk_ps[:])
        nc.vector.memset(qk_sb[:, 128:132], 1.0)
        attp[b] = dict(k1T=k1T, qk_sb=qk_sb)

    # ---------------- gate mix (all on PE) ----------------
    # h1_k = relu(sum_r m1[k,r]/r * B_r^expand + m1_b[k]);
    # gate = relu(sum_k m2[k] h1_k + m2_b).
    # Scaled-identity accumulating matmuls; bias folded into the ACT relu.
    # Expansion = stride-0 rhs views.
    gates = [None] * BPC
    for b in range(BPC):
        bl = bls[b]
        b1_sb = zob[b]["b1"]
        gate = []
        for dc in range(DC):
            p = DCH[dc]

            def ev_chunk(ri, t0, tn):
                r = RANGES[ri]
                if r == 1:
                    return b1_sb[dc][0:p, t0:t0 + tn]
                return bl[r][dc][0:p, t0 // r:(t0 + tn) // r, None] \
                    .to_broadcast([p, tn // r, r])

            h1 = []
            for k in range(3):
                acc = p_art.tile([128, T], DT, tag=f"artT{k}", name=f"h1_{k}")
                for t0 in range(0, T, N_MM):
                    ps = pp_g.tile([128, 512], F32, tag="g")
                    for ri in range(5):
                        nc.tensor.matmul(ps[0:p, :N_MM], mI[0:p, 5 * k + ri, 0:p],
                                         ev_chunk(ri, t0, N_MM),
                                         start=(ri == 0), stop=(ri == 4))
                    nc.scalar.activation(acc[0:p, t0:t0 + N_MM], ps[0:p, :N_MM],
                                         AF.Relu, bias=sc(SC_M1B + k, p))
                h1.append(acc)
            # gate combine on PE
            g_acc = p_mix.tile([128, T], DT, tag="gate")
            for t0 in range(0, T, N_MM):
                ps = pp_g.tile([128, 512], F32, tag="g")
                for k in range(3):
                    nc.tensor.matmul(ps[0:p, :N_MM], mI[0:p, 15 + k, 0:p],
                                     h1[k][0:p, t0:t0 + N_MM],
                                     start=(k == 0), stop=(k == 2))
                nc.scalar.activation(g_acc[0:p, t0:t0 + N_MM], ps[0:p, :N_MM],
                                     AF.Relu, bias=sc(SC_M2B, p))
            gate.append(g_acc)
        gates[b] = gate

    # ---------------- MRU scan + encode ----------------
    encs = [None] * BPC
    for b in range(BPC):
        gate = gates[b]
        z_sb = zob[b]["z"]
        o_sb = zob[b]["o"]
        encT = []
        for dc in range(DC):
            p = DCH[dc]
            gz = p_mix.tile([128, T], DT, tag="gzc", name="gz")
            nc.vector.tensor_mul(gz[0:p, :], gate[dc][0:p, :], z_sb[dc][0:p, :])
            nc.vector.tensor_sub(z_sb[dc][0:p, :], z_sb[dc][0:p, :], gz[0:p, :])
            c_t = p_mix.tile([128, T], DT, tag="gzc", name="c_t")
            nc.vector.tensor_tensor_scan(
                c_t[0:p, :], gate[dc][0:p, :], z_sb[dc][0:p, :], 0.0,
                op0=OP.mult, op1=OP.add)
            nc.vector.tensor_mul(o_sb[dc][0:p, :], o_sb[dc][0:p, :], c_t[0:p, :])
            encT.append(o_sb[dc])
        encs[b] = encT

    # ---------------- attention stream over 512-chunks ----------------
    pbs = [None] * BPC
    for b in range(BPC):
        encT = encs[b]
        k1T = attp[b]["k1T"]
        qk_sb = attp[b]["qk_sb"]
        pb_acc = persist.tile([128, 8], F32, tag=f"pb_acc{b}")
        for ci, (t0, tn) in enumerate(T512):
            s1 = pp_w.tile([TQ, 512], F32, tag="w")
            for dc in range(DC):
                nc.tensor.matmul(s1[:, :tn], k1T[0:DCH[dc], dc, :],
                                 encT[dc][0:DCH[dc], t0:t0 + tn],
                                 start=(dc == 0), stop=(dc == DC - 1))
            e1T = small.tile([TQ, 512], DT, tag="e1T")
            nc.scalar.activation(e1T[:, :tn], s1[:, :tn], AF.Exp)
            subs = TSUB if tn == 512 else TSUB_LAST
            pb_ps = pp_g.tile([128, 8], F32, tag="g")
            s0 = 0
            for pcs in subs:
                u2 = pp_w.tile([128, 132], F32, tag="w")
                nc.tensor.matmul(u2[:pcs, :], e1T[:, s0:s0 + pcs], qk_sb[:],
                                 start=True, stop=True)
                z1 = small.tile([128, 2], F32, tag="z1")
                nc.vector.reciprocal(z1[:pcs, 1:2], u2[:pcs, 128:129])
                e2 = small.tile([128, 128], F32, tag="e2")
                nc.scalar.activation(e2[:pcs, :], u2[:pcs, 0:128], AF.Exp,
                                     scale=z1[:pcs, 1:2])
                z2 = small.tile([128, 16], F32, tag="z2")
                nc.vector.tensor_reduce(
                    z2[:pcs, 0:8],
                    e2[:pcs, :].rearrange("p (g w) -> p g w", w=16),
                    AX.X, OP.add)
                nc.vector.reciprocal(z2[:pcs, 8:16], z2[:pcs, 0:8])
                nc.tensor.matmul(pb_ps[:, :], e2[:pcs, :], z2[:pcs, 8:16],
                                 start=(s0 == 0), stop=(s0 + pcs >= tn))
                s0 += pcs
            if ci == 0:
                nc.vector.tensor_copy(pb_acc[:], pb_ps[:])
            else:
                nc.vector.tensor_add(pb_acc[:], pb_acc[:], pb_ps[:])
        pbs[b] = pb_acc

    # ---------------- answer vectors ----------------
    for b in range(BPC):
        og = gathered[b]["og"]
        pb_sb = persist.tile([128, 8], DT, tag=f"pb_sb{b}")
        nc.vector.tensor_copy(pb_sb[:], pbs[b][:])
        ans_ps = pp_w.tile([128, 24], F32, tag="w")
        for g in range(8):
            fi, o = g // 4, g % 4
            pb16 = small.tile([TO, 1], DT, tag="pb16")
            nc.sync.dma_start(pb16[:], pb_sb[16 * g:16 * g + 16, g:g + 1])
            for dc in range(DC):
                j = fi * 3 + dc
                nc.tensor.matmul(ans_ps[0:DCH[dc], j * 4 + o:j * 4 + o + 1],
                                 og[o][:, DOFF[dc]:DOFF[dc] + DCH[dc]], pb16[:],
                                 start=True, stop=True)
        # 1/T of the mean-over-t lands here (cheaper than scaling rz2 per chunk)
        nc.vector.tensor_scalar_mul(
            ans_sb[:, b, :, :].rearrange("p j o -> p (j o)"), ans_ps[:], 1.0 / T)

    # ---------------- final MLP (both batches together) ----------------
    h_ps = pp_w.tile([75, 8], F32, tag="w")
    for j in range(6):
        # rhs columns = (b, o) pairs for chunk j of the 600-dim ans vector
        rhs = ans_sb[0:AS_SZ[j], :, j, :]
        nc.tensor.matmul(h_ps[:], fp[0:AS_SZ[j], F_AS1 + 75 * j:F_AS1 + 75 * (j + 1)],
                         rhs, start=(j == 0), stop=(j == 5))
    h_sb = small.tile([75, 8], F32, tag="h_sb")
    nc.scalar.activation(h_sb[:], h_ps[:], AF.Relu,
                         bias=fp[0:75, F_BAS1:F_BAS1 + 1])
    s_ps = pp_g.tile([8, 1], F32, tag="g")
    nc.tensor.matmul(s_ps[:], h_sb[:], fp[0:75, F_AS2:F_AS2 + 1],
                     start=True, stop=True)
    s_sb = small.tile([8, 1], F32, tag="s_sb")
    nc.scalar.activation(s_sb[:], s_ps[:], AF.Identity,
                         bias=fp[0:8, F_SCAL + SC_AS2B:F_SCAL + SC_AS2B + 1])
    nc.sync.dma_start(out[:].rearrange("b o -> (b o)")[:, None], s_sb[:])


# ---------------------------------------------------------------------------
# host side
# ---------------------------------------------------------------------------

_CACHE = {}


def _get_nc():
    if "nc" not in _CACHE:
        _CACHE["nc"] = _build_program()
    return _CACHE["nc"]


def _prep_core_inputs(inputs, core):
    b0 = core * BPC
    sl = slice(b0, b0 + BPC)
    f = np.asarray
    prep = _CACHE.get("prep_shared")
    if prep is None:
        # core-independent tensors, computed once per kernel() call set
        Wz, Wo = f(inputs["Wz"]), f(inputs["Wo"])
        ceW = f(inputs["ce_W"])
        w_art = np.concatenate([Wz.T, Wo.T, ceW[0].T], axis=1)   # [300, 900]
        w_ce = ceW[1:].transpose(0, 2, 1)                        # [4, 300, 300]
        w_f1 = f(inputs["f1_W"]).T
        # s2 = aoq @ f2W @ opt^T, so f2/f3 go in UNtransposed
        # (f1 builds keys1^T = f1W @ q^T and does need the transpose)
        w_f2 = f(inputs["f2_W"])
        w_f3 = f(inputs["f3_W"])
        wpack = np.zeros((128, WCOLS), np.float32)
        for kc in range(DC):
            rows = slice(DOFF[kc], DOFF[kc] + DCH[kc])
            n = DCH[kc]
            wpack[0:n, W_ART + kc * 900:W_ART + (kc + 1) * 900] = w_art[rows]
            for ri in range(4):
                o = W_CE + kc * 1200 + ri * 300
                wpack[0:n, o:o + 300] = w_ce[ri, rows]
            wpack[0:n, W_F1 + kc * 300:W_F1 + (kc + 1) * 300] = w_f1[rows]
            wpack[0:n, W_F2 + kc * 300:W_F2 + (kc + 1) * 300] = w_f2[rows]
            wpack[0:n, W_F3 + kc * 300:W_F3 + (kc + 1) * 300] = w_f3[rows]

        fpack = np.zeros((128, FCOLS), np.float32)
        biases = np.stack(
            [f(inputs["bz"]), f(inputs["bo"]),
             *[f(inputs["ce_b"])[i] for i in range(5)],
             f(inputs["f1_b"]), f(inputs["f2_b"]), f(inputs["f3_b"])],
            axis=1)                                              # [300, 10]
        for kc in range(DC):
            fpack[0:DCH[kc], F_BIAS + kc * 10:F_BIAS + (kc + 1) * 10] = \
                biases[DOFF[kc]:DOFF[kc] + DCH[kc]]
        m1 = f(inputs["mr1_W"])
        for k in range(3):
            for ri, r in enumerate(RANGES):
                fpack[:, F_SCAL + SC_M1 + 5 * k + ri] = m1[k, ri] / r
        fpack[:, F_SCAL + SC_M1B:F_SCAL + SC_M1B + 3] = f(inputs["mr1_b"])[None, :]
        fpack[:, F_SCAL + SC_M2:F_SCAL + SC_M2 + 3] = f(inputs["mr2_W"])[0][None, :]
        fpack[:, F_SCAL + SC_M2B] = f(inputs["mr2_b"])[0]
        fpack[:, F_SCAL + SC_AS2B] = f(inputs["as2_b"])[0]
        w_as1 = f(inputs["as1_W"]).T                             # [600, 75]
        r0 = 0
        for j in range(6):
            fpack[0:AS_SZ[j], F_AS1 + 75 * j:F_AS1 + 75 * (j + 1)] = \
                w_as1[r0:r0 + AS_SZ[j]]
            r0 += AS_SZ[j]
        fpack[0:75, F_AS2] = f(inputs["as2_W"])[0]
        fpack[0:75, F_BAS1] = f(inputs["as1_b"])

        prep = {
            "emb": f(inputs["emb"]).astype(NPDT),
            "wpack": wpack.astype(NPDT),
            "fpack": fpack,
        }
        _CACHE["prep_shared"] = prep

    d = dict(prep)
    art = f(inputs["article_in"])[sl].astype(np.int32)
    q = f(inputs["question_in"])[sl].astype(np.int32)
    ixp = np.zeros((128, BPC, IXCOLS), np.int32)
    for b in range(BPC):
        for c in range(15):
            ixp[:, b, IX_ART + c] = art[b, c * 128:(c + 1) * 128]
        ixp[0:80, b, IX_ART + 15] = art[b, 1920:2000]
        ixp[0:TQ, b, IX_Q] = q[b]
        for o in range(4):
            ixp[0:TO, b, IX_OPT + o] = \
                f(inputs[f"option{o + 1}_in"])[sl][b].astype(np.int32)
    d["idx_pack"] = ixp
    return d


def run_cores(per_core_inputs, trace=False):
    """per_core_inputs: list of 8 dicts name->np array. Returns results."""
    from concourse import bass_utils
    nc = _get_nc()
    return bass_utils.run_bass_kernel_spmd(
        nc, per_core_inputs, core_ids=list(range(NCORES)),
        trace=trace, trace_cores=[0] if trace else None)


def kernel(**inputs):
    _CACHE.pop("prep_shared", None)
    per_core = [_prep_core_inputs(inputs, c) for c in range(NCORES)]
    res = run_cores(per_core)
    out = np.concatenate([res.results[c]["scores"] for c in range(NCORES)],
                         axis=0)
    return out.astype(np.float32)
